# revision 3
# baseline (speedup 1.0000x reference)
"""Bass TRN2 kernel for nn_MAFG_CA (sparse window attention), 8-core row-band sharding.

Self-contained: builds one raw-Bass program (manual semaphores via a small
dependency tracker), compiles once via bass_utils/bass2jax on the 8 axon
NeuronCores, reassembles the full output on host.
"""
import sys
sys.path.insert(0, "/opt/trn_rl_repo")

import numpy as np
import ml_dtypes
import concourse.bass as bass
from concourse import mybir

M = 5
WS = 8
OW = 12
PAD = 2
NH = 8
E = 64
HD = E // NH
H = W = 128
NCORES = 8
BAND = H // NCORES        # 16
RH = BAND + 2 * PAD       # 20
WC = W // WS              # 16
WR = BAND // WS           # 2
NSP = WR * WC             # 32
PIX = M * RH * W          # 12800
CW = W + 2 * PAD          # 132
SCALE = float(HD) ** -0.5
F32 = mybir.dt.float32
BF16 = mybir.dt.bfloat16
KA = 120
KB = 24
AOP = mybir.AluOpType
AF = mybir.ActivationFunctionType


class Sched:
    def __init__(self):
        self.ops = {k: [] for k in ("pe", "act", "dve", "pool", "sp")}
        self.cnt = {k: 0 for k in ("pe", "act", "dve", "pool", "sp", "dma", "dmout")}
        self.bufs = {}

    def _b(self, n):
        return self.bufs.setdefault(n, {"w": None, "r": []})

    def emit(self, eng, fn, reads=(), writes=(), dma=False, dma_sem="dma"):
        waits = {}

        def add(tok):
            if tok is None:
                return
            s, v = tok
            if s == eng:
                return
            waits[s] = max(waits.get(s, 0), v)

        for b in reads:
            add(self._b(b)["w"])
        for b in writes:
            st = self._b(b)
            add(st["w"])
            for tok in st["r"]:
                add(tok)
        if dma:
            self.cnt[dma_sem] += 16
            tok = (dma_sem, self.cnt[dma_sem])
        else:
            self.cnt[eng] += 1
            tok = (eng, self.cnt[eng])
        for b in reads:
            self._b(b)["r"].append(tok)
        for b in writes:
            st = self._b(b)
            st["w"] = tok
            st["r"] = []
        self.ops[eng].append((dict(waits), fn))

    def wait_all(self, eng):
        snap = {k: self.cnt[k] for k in self.cnt if self.cnt[k] > 0 and k != eng}
        self.ops[eng].append((snap, None))


def build_kernel():
    nc = bass.Bass("TRN2")

    xin = nc.dram_tensor("x", [M, 3, RH, W], BF16, kind="ExternalInput")
    d_whT = nc.dram_tensor("whT", [3, E], BF16, kind="ExternalInput")
    d_bh = nc.dram_tensor("bh", [E, 1], F32, kind="ExternalInput")
    d_wqg = nc.dram_tensor("wqg", [E, E], BF16, kind="ExternalInput")
    d_wkg = nc.dram_tensor("wkg", [E, E], BF16, kind="ExternalInput")
    d_wva = nc.dram_tensor("wva", [E + 1, E + 1], BF16, kind="ExternalInput")
    d_posq = nc.dram_tensor("posq", [E, WS * WS], BF16, kind="ExternalInput")
    d_posk = nc.dram_tensor("posk", [E, OW * OW], BF16, kind="ExternalInput")
    d_wout = nc.dram_tensor("woutT", [E, E], BF16, kind="ExternalInput")
    d_wrgb = nc.dram_tensor("wrgbA", [E + 1, 3], BF16, kind="ExternalInput")
    d_i64 = nc.dram_tensor("i64", [E, E], BF16, kind="ExternalInput")
    d_i128 = nc.dram_tensor("i128", [128, 128], BF16, kind="ExternalInput")
    d_i100 = nc.dram_tensor("i100", [100, 100], F32, kind="ExternalInput")
    yout = nc.dram_tensor("y", [M, 3, BAND, W], F32, kind="ExternalOutput")

    S = Sched()
    import contextlib
    ctx = contextlib.ExitStack()
    with ctx:
        sems = {k: ctx.enter_context(nc.semaphore())
                for k in ("pe", "act", "dve", "pool", "dma", "dmout")}

        sb = lambda shape, dt: ctx.enter_context(nc.sbuf_tensor(shape, dt))
        xsb = sb([3, PIX], BF16)
        whT = sb([3, E], BF16)
        bh = sb([E, 1], F32)
        wqg = sb([E, E], BF16)
        wkg = sb([E, E], BF16)
        wva = sb([E + 1, E + 1], BF16)
        posq = sb([E, WS * WS], BF16)
        posk = sb([E, OW * OW], BF16)
        woutT = sb([E, E], BF16)
        wrgbA = sb([E + 1, 3], BF16)
        i64 = sb([E, E], BF16)
        i128 = sb([128, 128], BF16)
        i100 = sb([100, 100], F32)
        onesmu = sb([E, 1], BF16)
        eps100 = sb([100, 1], F32)
        zero120 = sb([KA, 1], F32)

        featEm = sb([E, PIX], BF16)
        sqrot = sb([E, 4, 512], BF16)
        mu_t = sb([100, W], F32)
        var_t = sb([100, W], F32)
        std_t = sb([100, W], F32)
        rstd_t = sb([100, W], F32)
        tmt = sb([100, W], F32)
        rt = sb([128, 200], F32)
        lng = sb([128, 4 * E], F32)
        xspm = sb([128, 100 * E], BF16)
        xsEm = sb([E + 1, M * RH * CW], BF16)
        QEm = sb([E, RH * W], BF16)
        KEm = sb([E, M * RH * CW], BF16)

        qeff = [sb([E, WS * WS], BF16) for _ in range(2)]
        keff = [sb([E, M * OW * OW], BF16) for _ in range(2)]
        Qp = [sb([E, 512], BF16) for _ in range(3)]
        PA = [[sb([KA, 1024], BF16) for _ in range(3)] for _ in range(2)]
        Vw = [sb([KA, M * 65 + 65], BF16) for _ in range(2)]
        attnsb = [sb([128, 160], BF16) for _ in range(2)]
        recs = [sb([128, 20], F32) for _ in range(2)]
        attnT = sb([E, NSP * M * WS * WS], BF16)
        mixA = sb([E + 1, WR * M * WS * W], BF16)
        rgbb = [sb([3, 512], F32) for _ in range(2)]

        ps = lambda shape: ctx.enter_context(nc.psum_tensor(shape, F32))
        pA = ps([128, 512])
        pB = ps([128, 512])
        pC = ps([128, 512])
        pD = ps([128, 512])
        pO = ps([128, 1300])
        pV = ps([KA, 400])
        PN = {id(pA): "pA", id(pB): "pB", id(pC): "pC", id(pD): "pD",
              id(pO): "pO", id(pV): "pV"}

        OFF_B = M * 65

        # ============ loads + memsets ============
        loads = [(xsb[:, :], xin.rearrange("f c r w -> c (f r w)")),
                 (whT[:, :], d_whT[:, :]), (bh[:, :], d_bh[:, :]),
                 (wqg[:, :], d_wqg[:, :]), (wkg[:, :], d_wkg[:, :]),
                 (wva[:, :], d_wva[:, :]), (posq[:, :], d_posq[:, :]),
                 (posk[:, :], d_posk[:, :]), (woutT[:, :], d_wout[:, :]),
                 (wrgbA[:, :], d_wrgb[:, :]), (i64[:, :], d_i64[:, :]),
                 (i128[:, :], d_i128[:, :]), (i100[:, :], d_i100[:, :])]
        for idx, (dst, src) in enumerate(loads):
            S.emit("sp", lambda eng, dst=dst, src=src: nc.sync.dma_start(
                dst, src).then_inc(sems["dma"], 16),
                dma=True, writes=[f"ld{idx}"])

        S.emit("dve", lambda eng: nc.vector.memset(onesmu[:, :], 1.0 / E).then_inc(sems["dve"], 1), writes=["cst"])
        S.emit("dve", lambda eng: nc.vector.memset(eps100[:, :], 1e-6).then_inc(sems["dve"], 1), writes=["cst"])
        S.emit("dve", lambda eng: nc.vector.memset(zero120[:, :], 0.0).then_inc(sems["dve"], 1), writes=["cst"])
        S.emit("dve", lambda eng: nc.vector.memset(xsEm[:, :], 0.0).then_inc(sems["dve"], 1), writes=["xsEm"])
        S.emit("dve", lambda eng: nc.vector.memset(KEm[:, :], 0.0).then_inc(sems["dve"], 1), writes=["KEm"])
        S.emit("pool", lambda eng: nc.gpsimd.memset(xsEm[E:E + 1, :], 1.0).then_inc(sems["pool"], 1), writes=["xsEm"])
        S.emit("pool", lambda eng: nc.gpsimd.memset(mixA[E:E + 1, :], 1.0).then_inc(sems["pool"], 1), writes=["mixA"])
        for i in range(3):
            S.emit("pool", lambda eng, i=i: nc.gpsimd.memset(Qp[i][:, :], 0.0).then_inc(sems["pool"], 1),
                   writes=[f"Qp{i}h{h}" for h in range(NH)])

        for e in ("pe", "dve", "act", "pool"):
            S.wait_all(e)

        # ============ phase 1: feat ============
        for i in range(25):
            p = (pA, pB)[i % 2]
            t = PN[id(p)]
            S.emit("pe", lambda eng, i=i, p=p: nc.tensor.matmul(
                p[0:E, :], whT[:, :], xsb[:, i * 512:(i + 1) * 512],
                start=True, stop=True).then_inc(sems["pe"], 1),
                reads=["ld0", "ld1"], writes=[t])
            S.emit("pool", lambda eng, i=i, p=p: nc.gpsimd.tensor_scalar(
                featEm[:, i * 512:(i + 1) * 512], p[0:E, :], bh[:, :], None,
                AOP.add).then_inc(sems["pool"], 1),
                reads=[t, "ld2"], writes=["featEm", t])

        # ============ phase 2: LN stats ============
        for i in range(25):
            S.emit("dve", lambda eng, i=i: nc.vector.tensor_mul(
                sqrot[:, i % 4, :], featEm[:, i * 512:(i + 1) * 512],
                featEm[:, i * 512:(i + 1) * 512]).then_inc(sems["dve"], 1),
                reads=["featEm"], writes=[f"sq{i % 4}"])
            for j in range(4):
                row = i * 4 + j
                S.emit("pe", lambda eng, row=row: nc.tensor.matmul(
                    pC[row:row + 1, 0:W], onesmu[:, :],
                    featEm[:, row * W:(row + 1) * W],
                    start=True, stop=True).then_inc(sems["pe"], 1),
                    reads=["featEm", "cst"], writes=["pC"])
                S.emit("pe", lambda eng, row=row, i=i, j=j: nc.tensor.matmul(
                    pC[row:row + 1, W:2 * W], onesmu[:, :],
                    sqrot[:, i % 4, j * W:(j + 1) * W],
                    start=True, stop=True).then_inc(sems["pe"], 1),
                    reads=[f"sq{i % 4}", "cst"], writes=["pC"])
        S.emit("dve", lambda eng: nc.vector.tensor_mul(
            var_t[:, :], pC[0:100, 0:W], pC[0:100, 0:W]).then_inc(sems["dve"], 1),
            reads=["pC"], writes=["var"])
        S.emit("dve", lambda eng: nc.vector.tensor_tensor(
            var_t[:, :], pC[0:100, W:2 * W], var_t[:, :],
            AOP.subtract).then_inc(sems["dve"], 1),
            reads=["pC", "var"], writes=["var"])
        S.emit("act", lambda eng: nc.scalar.activation(
            std_t[:, :], var_t[:, :], AF.Sqrt,
            bias=eps100[:, :]).then_inc(sems["act"], 1),
            reads=["var", "cst"], writes=["std"])
        S.emit("dve", lambda eng: nc.vector.reciprocal(
            rstd_t[:, :], std_t[:, :]).then_inc(sems["dve"], 1),
            reads=["std"], writes=["rstd"])
        S.emit("dve", lambda eng: nc.vector.tensor_scalar(
            mu_t[:, :], pC[0:100, 0:W], 1.0, None, AOP.mult).then_inc(sems["dve"], 1),
            reads=["pC"], writes=["mu"])
        S.emit("dve", lambda eng: nc.vector.tensor_mul(
            tmt[:, :], mu_t[:, :], rstd_t[:, :]).then_inc(sems["dve"], 1),
            reads=["mu", "rstd"], writes=["tm"])
        S.emit("pe", lambda eng: nc.tensor.matmul(
            pD[0:128, 0:100], rstd_t[:, :], i100[:, :],
            start=True, stop=True).then_inc(sems["pe"], 1),
            reads=["rstd", "ld12"], writes=["pD"])
        S.emit("pe", lambda eng: nc.tensor.matmul(
            pD[0:128, 100:200], tmt[:, :], i100[:, :],
            start=True, stop=True).then_inc(sems["pe"], 1),
            reads=["tm", "ld12"], writes=["pD"])
        S.emit("dve", lambda eng: nc.vector.tensor_scalar(
            rt[:, :], pD[0:128, 0:200], 1.0, None, AOP.mult).then_inc(sems["dve"], 1),
            reads=["pD"], writes=["rt"])

        # ============ phase 3: feat^T chunks + LN apply -> xs pixel-major ============
        for g in range(25):
            p = (pA, pB)[g % 2]
            t = PN[id(p)]
            for j in range(4):
                c = g * 4 + j
                S.emit("pe", lambda eng, c=c, j=j, p=p: nc.tensor.matmul(
                    p[0:128, j * E:(j + 1) * E],
                    featEm[:, c * 128:(c + 1) * 128], i64[:, :],
                    start=True, stop=True).then_inc(sems["pe"], 1),
                    reads=["featEm", "ld10"], writes=[t])
            rb = bass.AP(tensor=rt.tensor, offset=rt[:, 4 * g:4 * g + 4].offset,
                         ap=[[1, 128], [1, 4], [0, E]])
            tb = bass.AP(tensor=rt.tensor, offset=rt[:, 100 + 4 * g:100 + 4 * g + 4].offset,
                         ap=[[1, 128], [1, 4], [0, E]])
            S.emit("dve", lambda eng, p=p, rb=rb: nc.vector.tensor_tensor(
                lng[:, :], p[0:128, 0:4 * E], rb, AOP.mult).then_inc(sems["dve"], 1),
                reads=[t, "rt"], writes=["lng", t])
            S.emit("dve", lambda eng, g=g, tb=tb: nc.vector.tensor_tensor(
                xspm[:, g * 4 * E:(g + 1) * 4 * E], lng[:, :], tb,
                AOP.subtract).then_inc(sems["dve"], 1),
                reads=["lng", "rt"], writes=["xspm"])

        # ============ phase 4: xs -> xsEm (transpose back, padded) ============
        for g in range(25):
            p = (pC, pD)[g % 2]
            t = PN[id(p)]
            for j in range(4):
                c = g * 4 + j
                S.emit("pe", lambda eng, c=c, j=j, p=p: nc.tensor.matmul(
                    p[0:E, j * 128:(j + 1) * 128],
                    xspm[:, c * E:(c + 1) * E], i128[:, :],
                    start=True, stop=True).then_inc(sems["pe"], 1),
                    reads=["xspm", "ld11"], writes=[t])
            fr = g * 4
            dst = bass.AP(tensor=xsEm.tensor, offset=xsEm[0:E, fr * CW + PAD:].offset,
                          ap=[[1, E], [CW, 4], [1, W]])
            S.emit("pool", lambda eng, dst=dst, p=p: nc.gpsimd.tensor_copy(
                out=dst, in_=p[0:E, 0:512]).then_inc(sems["pool"], 1),
                reads=[t], writes=["xsEm", t])

        # ============ phase 5: K (all frames) + Q (frame 2) ============
        for i in range(25):
            p = (pA, pB)[i % 2]
            t = PN[id(p)]
            fr = i * 4
            src = bass.AP(tensor=xsEm.tensor, offset=xsEm[0:E, fr * CW + PAD:].offset,
                          ap=[[1, E], [CW, 4], [1, W]])
            dstk = bass.AP(tensor=KEm.tensor, offset=KEm[0:E, fr * CW + PAD:].offset,
                           ap=[[1, E], [CW, 4], [1, W]])
            S.emit("pe", lambda eng, p=p, src=src: nc.tensor.matmul(
                p[0:E, 0:512], wkg[:, :], src,
                start=True, stop=True).then_inc(sems["pe"], 1),
                reads=["xsEm", "ld4"], writes=[t])
            S.emit("act", lambda eng, p=p, dstk=dstk: nc.scalar.activation(
                dstk, p[0:E, 0:512], AF.Copy).then_inc(sems["act"], 1),
                reads=[t], writes=["KEm", t])
        for i in range(5):
            p = (pC, pD)[i % 2]
            t = PN[id(p)]
            fr = 2 * RH + i * 4
            src = bass.AP(tensor=xsEm.tensor, offset=xsEm[0:E, fr * CW + PAD:].offset,
                          ap=[[1, E], [CW, 4], [1, W]])
            S.emit("pe", lambda eng, p=p, src=src: nc.tensor.matmul(
                p[0:E, 0:512], wqg[:, :], src,
                start=True, stop=True).then_inc(sems["pe"], 1),
                reads=["xsEm", "ld3"], writes=[t])
            S.emit("act", lambda eng, i=i, p=p: nc.scalar.activation(
                QEm[:, i * 512:(i + 1) * 512], p[0:E, 0:512],
                AF.Copy).then_inc(sems["act"], 1),
                reads=[t], writes=["QEm", t])

        # ============ phase 6: attention ============
        for w in range(NSP):
            wr, wc = divmod(w, WC)
            r2 = w % 2
            r3 = w % 3
            kbox = bass.AP(tensor=KEm.tensor,
                           offset=KEm[0:E, (wr * WS) * CW + wc * WS:].offset,
                           ap=[[1, E], [RH * CW, M], [CW, OW], [1, OW]])
            pk5 = bass.AP(tensor=posk.tensor, offset=posk[:, :].offset,
                          ap=[[1, E], [0, M], [1, OW * OW]])
            S.emit("dve", lambda eng, kbox=kbox, pk5=pk5, r2=r2: nc.vector.tensor_tensor(
                keff[r2][:, :], kbox, pk5, AOP.add).then_inc(sems["dve"], 1),
                reads=["KEm", "ld7"], writes=[f"keff{r2}"])
            qbox = bass.AP(tensor=QEm.tensor,
                           offset=QEm[0:E, (PAD + wr * WS) * W + wc * WS:].offset,
                           ap=[[1, E], [W, WS], [1, WS]])
            S.emit("dve", lambda eng, qbox=qbox, r2=r2: nc.vector.tensor_add(
                qeff[r2][:, :], qbox, posq[:, :]).then_inc(sems["dve"], 1),
                reads=["QEm", "ld6"], writes=[f"qeff{r2}"])
            for h in range(NH):
                S.emit("sp", lambda eng, h=h, r2=r2, r3=r3: nc.sync.dma_start(
                    Qp[r3][h * HD:(h + 1) * HD, h * 64:(h + 1) * 64],
                    qeff[r2][h * HD:(h + 1) * HD, :]).then_inc(sems["dma"], 16),
                    dma=True, reads=[f"qeff{r2}"], writes=[f"Qp{r3}h{h}"])
            # V windows
            for f in range(M):
                vbox = bass.AP(tensor=xsEm.tensor,
                               offset=xsEm[0:E + 1, f * RH * CW + (wr * WS) * CW + wc * WS:].offset,
                               ap=[[1, E + 1], [CW, 10], [1, OW]])
                S.emit("pe", lambda eng, vbox=vbox, f=f: nc.tensor.matmul(
                    pV[0:KA, f * 65:(f + 1) * 65], vbox, wva[:, :],
                    start=True, stop=True).then_inc(sems["pe"], 1),
                    reads=["xsEm", "ld5"], writes=["pV"])
            vboxb = bass.AP(tensor=xsEm.tensor,
                            offset=xsEm[0:E + 1, (wr * WS + 10) * CW + wc * WS:].offset,
                            ap=[[1, E + 1], [RH * CW, M], [CW, 2], [1, OW]])
            S.emit("pe", lambda eng, vboxb=vboxb: nc.tensor.matmul(
                pV[0:KA, OFF_B:OFF_B + 65], vboxb, wva[:, :],
                start=True, stop=True).then_inc(sems["pe"], 1),
                reads=["xsEm", "ld5"], writes=["pV"])
            S.emit("pool", lambda eng, r2=r2: nc.gpsimd.tensor_copy(
                out=Vw[r2][:, 0:OFF_B + 65],
                in_=pV[0:KA, 0:OFF_B + 65]).then_inc(sems["pool"], 1),
                reads=["pV"], writes=[f"Vw{r2}", "pV"])

            qreads = [f"Qp{r3}h{h}" for h in range(NH)]

            def st_mm(dst, lhs, tag, r2=r2, r3=r3, qreads=qreads):
                S.emit("pe", lambda eng, dst=dst, lhs=lhs, r3=r3: nc.tensor.matmul(
                    dst, lhs, Qp[r3][:, :],
                    start=True, stop=True).then_inc(sems["pe"], 1),
                    reads=[f"keff{r2}"] + qreads, writes=[tag])

            def expop(dstap, srcap, dsttag, srctag):
                S.emit("act", lambda eng, dstap=dstap, srcap=srcap: nc.scalar.activation(
                    dstap, srcap, AF.Exp, bias=zero120[:, :],
                    scale=SCALE).then_inc(sems["act"], 1),
                    reads=[srctag, "cst"], writes=[dsttag, srctag])

            st_mm(pA[0:KA, 0:512], keff[r2][:, 0 * 144:0 * 144 + KA], "pA")
            st_mm(pB[0:KA, 0:512], keff[r2][:, 1 * 144:1 * 144 + KA], "pB")
            st_mm(pC[0:KA, 0:512], keff[r2][:, 2 * 144:2 * 144 + KA], "pC")
            st_mm(pD[0:KA, 0:512], keff[r2][:, 3 * 144:3 * 144 + KA], "pD")
            expop(PA[r2][0][:, 0:512], pA[0:KA, 0:512], f"PA0_{r2}", "pA")
            expop(PA[r2][0][:, 512:1024], pB[0:KA, 0:512], f"PA0_{r2}", "pB")
            st_mm(pA[0:KA, 0:512], keff[r2][:, 4 * 144:4 * 144 + KA], "pA")
            bap = bass.AP(tensor=keff[r2].tensor, offset=keff[r2][:, KA:].offset,
                          ap=[[1, E], [144, M], [1, KB]])
            st_mm(pB[0:KB * M, 0:512], bap, "pB")
            expop(PA[r2][1][:, 0:512], pC[0:KA, 0:512], f"PA1_{r2}", "pC")
            expop(PA[r2][1][:, 512:1024], pD[0:KA, 0:512], f"PA1_{r2}", "pD")
            expop(PA[r2][2][:, 0:512], pA[0:KA, 0:512], f"PA2_{r2}", "pA")
            expop(PA[r2][2][:, 512:1024], pB[0:KB * M, 0:512], f"PA2_{r2}", "pB")

            FMAP = [(0, 0), (0, 1), (1, 0), (1, 1), (2, 0)]
            for f in range(M):
                pi, half = FMAP[f]
                for c in range(4):
                    lhs_a = PA[r2][pi][0:KA, half * 512 + c * 128:half * 512 + (c + 1) * 128]
                    rhs_a = Vw[r2][0:KA, f * 65:f * 65 + 64]
                    one_a = Vw[r2][0:KA, f * 65 + 64:f * 65 + 65]
                    lhs_b = PA[r2][2][f * KB:(f + 1) * KB, 512 + c * 128:512 + (c + 1) * 128]
                    rhs_b = Vw[r2][f * KB:(f + 1) * KB, OFF_B:OFF_B + 64]
                    one_b = Vw[r2][f * KB:(f + 1) * KB, OFF_B + 64:OFF_B + 65]
                    oc = c * 320 + f * 64
                    dc = 1280 + c * 5 + f
                    for (lh, rh, o0, o1, st0, st1) in (
                            (lhs_a, rhs_a, oc, oc + 64, True, False),
                            (lhs_a, one_a, dc, dc + 1, True, False),
                            (lhs_b, rhs_b, oc, oc + 64, False, True),
                            (lhs_b, one_b, dc, dc + 1, False, True)):
                        ptag = f"PA{pi}_{r2}" if lh is lhs_a else f"PA2_{r2}"
                        S.emit("pe", lambda eng, lh=lh, rh=rh, o0=o0, o1=o1,
                               st0=st0, st1=st1: nc.tensor.matmul(
                                   pO[0:128, o0:o1], lh, rh, start=st0, stop=st1,
                                   skip_group_check=True).then_inc(sems["pe"], 1),
                               reads=[ptag, f"Vw{r2}"], writes=["pO"])

            S.emit("dve", lambda eng, r2=r2: nc.vector.reciprocal(
                recs[r2][:, :], pO[0:128, 1280:1300]).then_inc(sems["dve"], 1),
                reads=["pO"], writes=[f"rec{r2}"])
            for hl in range(2):
                src = bass.AP(tensor=pO.tensor, offset=pO[hl * 64:, 8 * hl:].offset,
                              ap=[[1, 64], [336, 4], [64, M], [1, 8]])
                rmul = bass.AP(tensor=recs[r2].tensor, offset=recs[r2][hl * 64:, :].offset,
                               ap=[[1, 64], [1, 20], [0, 8]])
                dst = bass.AP(tensor=attnsb[r2].tensor, offset=attnsb[r2][hl * 64:, :].offset,
                              ap=[[1, 64], [40, 4], [8, M], [1, 8]])
                S.emit("dve", lambda eng, src=src, rmul=rmul, dst=dst, r2=r2:
                       nc.vector.tensor_tensor(dst, src, rmul, AOP.mult).then_inc(sems["dve"], 1),
                       reads=["pO", f"rec{r2}"], writes=[f"attnsb{r2}", "pO"])

            for c in range(4):
                for hl in range(2):
                    hd0 = (2 * c + hl) * 8
                    for f in range(M):
                        lhs = bass.AP(tensor=attnsb[r2].tensor,
                                      offset=attnsb[r2][hl * 64:, c * 40 + f * 8:].offset,
                                      ap=[[1, 64], [1, 8]])
                        S.emit("pe", lambda eng, lhs=lhs, hd0=hd0, f=f: nc.tensor.matmul(
                            pV[hd0:hd0 + 8, f * 64:(f + 1) * 64], lhs, i64[:, :],
                            start=True, stop=True).then_inc(sems["pe"], 1),
                            reads=[f"attnsb{r2}", "ld10"], writes=["pV"])
            dstT = bass.AP(tensor=attnT.tensor,
                           offset=attnT[0:E, wr * (M * WS * W) + wc * WS:].offset,
                           ap=[[1, E], [WS * W, M], [W, WS], [1, WS]])
            srcT = bass.AP(tensor=pV.tensor, offset=pV[0:E, 0:].offset,
                           ap=[[1, E], [64, M], [8, WS], [1, WS]])
            S.emit("pool", lambda eng, dstT=dstT, srcT=srcT: nc.gpsimd.tensor_copy(
                out=dstT, in_=srcT).then_inc(sems["pool"], 1),
                reads=["pV"], writes=["attnT", "pV"])

        # ============ phase 7: out-proj + residual + rgb + store ============
        for i in range(20):
            p = (pA, pB)[i % 2]
            t = PN[id(p)]
            S.emit("pe", lambda eng, i=i, p=p: nc.tensor.matmul(
                p[0:E, 0:512], woutT[:, :], attnT[:, i * 512:(i + 1) * 512],
                start=True, stop=True).then_inc(sems["pe"], 1),
                reads=["attnT", "ld8"], writes=[t])
            r0 = i * 4
            wrr, rem = divmod(r0, 40)
            ff, qr = divmod(rem, 8)
            fap = bass.AP(tensor=featEm.tensor,
                          offset=featEm[0:E, ff * RH * W + (PAD + wrr * WS + qr) * W:].offset,
                          ap=[[1, E], [W, 4], [1, W]])
            S.emit("dve", lambda eng, i=i, p=p, fap=fap: nc.vector.tensor_tensor(
                mixA[0:E, i * 512:(i + 1) * 512], p[0:E, 0:512], fap,
                AOP.add).then_inc(sems["dve"], 1),
                reads=[t, "featEm"], writes=["mixA", t])
        for i in range(20):
            p = (pC, pD)[i % 2]
            t = PN[id(p)]
            S.emit("pe", lambda eng, i=i, p=p: nc.tensor.matmul(
                p[0:3, 0:512], wrgbA[:, :], mixA[:, i * 512:(i + 1) * 512],
                start=True, stop=True).then_inc(sems["pe"], 1),
                reads=["mixA", "ld9"], writes=[t])
            S.emit("act", lambda eng, i=i, p=p: nc.scalar.activation(
                rgbb[i % 2][:, :], p[0:3, 0:512], AF.Copy).then_inc(sems["act"], 1),
                reads=[t], writes=[f"rgbb{i % 2}", t])
            r0 = i * 4
            wrr, rem = divmod(r0, 40)
            ff, qr = divmod(rem, 8)
            dst = bass.AP(tensor=yout.tensor,
                          offset=yout[ff, 0:3, wrr * WS + qr, 0:].offset,
                          ap=[[BAND * W, 3], [W, 4], [1, W]])
            S.emit("sp", lambda eng, dst=dst, i=i: nc.sync.dma_start(
                dst, rgbb[i % 2][:, :]).then_inc(sems["dmout"], 16),
                dma=True, dma_sem="dmout", reads=[f"rgbb{i % 2}"], writes=[f"y{i}"])

        S.wait_all("sp")

        with nc.Block() as block:
            def mk(engname):
                def body(eng):
                    for waits, fn in S.ops[engname]:
                        for s, v in sorted(waits.items()):
                            eng.wait_ge(sems[s], v)
                        if fn is not None:
                            fn(eng)
                return body
            block.sync(mk("sp"))
            block.tensor(mk("pe"))
            block.scalar(mk("act"))
            block.vector(mk("dve"))
            block.gpsimd(mk("pool"))

    return nc


_CACHE = {}


def _prep_inputs(inputs):
    bf = ml_dtypes.bfloat16
    x = np.asarray(inputs["x"], np.float32).reshape(M, 3, H, W)
    gamma = np.asarray(inputs["gamma"], np.float32)
    beta = np.asarray(inputs["beta"], np.float32)
    w_qkv = np.asarray(inputs["w_qkv"], np.float32)
    Wq, Wk, Wv = w_qkv[0:E], w_qkv[E:2 * E], w_qkv[2 * E:3 * E]
    WqG = (Wq * gamma[None, :]).T.copy()
    WkG = (Wk * gamma[None, :]).T.copy()
    WvG = (Wv * gamma[None, :]).T.copy()
    bq = beta @ Wq.T
    bk = beta @ Wk.T
    bv = beta @ Wv.T
    pos_q = np.asarray(inputs["pos_q"], np.float32)
    pos_k = np.asarray(inputs["pos_k"], np.float32)
    posqT = pos_q.T + bq[:, None]
    poskT = pos_k.T + bk[:, None]
    w_out = np.asarray(inputs["w_out"], np.float32)
    w_rgb = np.asarray(inputs["w_rgb"], np.float32)
    b_rgb = np.asarray(inputs["b_rgb"], np.float32)
    b_rgb_eff = b_rgb + w_rgb @ (w_out @ bv)
    wva = np.zeros((E + 1, E + 1), np.float32)
    wva[0:E, 0:E] = WvG
    wva[E, E] = 1.0
    wrgbA = np.zeros((E + 1, 3), np.float32)
    wrgbA[0:E, :] = w_rgb.T
    wrgbA[E, :] = b_rgb_eff
    com = {
        "whT": np.asarray(inputs["w_high"], np.float32).T.astype(bf),
        "bh": np.asarray(inputs["b_high"], np.float32).reshape(E, 1),
        "wqg": WqG.astype(bf), "wkg": WkG.astype(bf), "wva": wva.astype(bf),
        "posq": posqT.astype(bf), "posk": poskT.astype(bf),
        "woutT": w_out.T.copy().astype(bf),
        "wrgbA": wrgbA.astype(bf),
        "i64": np.eye(E, dtype=np.float32).astype(bf),
        "i128": np.eye(128, dtype=np.float32).astype(bf),
        "i100": np.eye(100, dtype=np.float32),
    }
    in_maps = []
    for ci in range(NCORES):
        r0 = ci * BAND - PAD
        xb = np.zeros((M, 3, RH, W), np.float32)
        lo, hi = max(r0, 0), min(r0 + RH, H)
        xb[:, :, lo - r0:hi - r0, :] = x[:, :, lo:hi, :]
        m = dict(com)
        m["x"] = xb.astype(bf)
        in_maps.append(m)
    return in_maps


def kernel(**inputs):
    if "nc" not in _CACHE:
        from concourse import bass_utils
        _CACHE["nc"] = build_kernel()
        _CACHE["bu"] = bass_utils
    nc = _CACHE["nc"]
    in_maps = _prep_inputs(inputs)
    res = _CACHE["bu"].run_bass_kernel_spmd(nc, in_maps, core_ids=list(range(NCORES)))
    out = np.stack([res.results[ci]["y"] for ci in range(NCORES)], axis=0)
    full = out.transpose(1, 2, 0, 3, 4).reshape(M, 3, H, W)
    return full.reshape(1, M * 3, H, W).astype(np.float32)


if __name__ == "__main__":
    d = np.load("/root/problem/expected.npz")
    inputs = {k[3:]: d[k] for k in d.files if k.startswith("in_")}
    expected = d["expected"]
    actual = kernel(**inputs)
    rel = np.abs(actual - expected) / (np.abs(expected).max() + 1e-8)
    print("rel err:", float(rel.max()))
